# revision 6
# baseline (speedup 1.0000x reference)
"""Trainium2 Bass kernel for windowed (block-sparse) attention encoder.

Model (reference):
  q/k/v = 1x1 conv projections of x1 [B,C,S] with weights [E,C]
  queries split into nb = S/D blocks of D tokens; k/v use overlapping
  windows of width 2D (stride D, halo D/2 each side, zero-padded)
  attn = softmax(qk/sqrt(E) + log(fmask+1e-6)) * fmask
  y = Wo @ gelu(attn @ v) + bo, masked by the padding mask.

Sharding: 8 cores = batch (4) x sequence halves (2). Each core gets a
halo'd x slice [C, S/2 + D] so no cross-core communication is needed.

Device layout (per core):
  qw/kw: [E(part), token]   energyT[j,q] = kw^T qw  (j on partitions)
  vT:    [token(part), E]   av[e,q] = vT^T s2
  softmax runs along the partition (j) dim with no max subtraction
  (energies are O(+-10), exp is fp32-safe); the window/padding mask
  enters as a per-partition log-bias on the exp and a per-partition
  multiplier, so no partition-broadcasts are needed anywhere.
All matmul operands use float32r (fast fp32 path on the PE).
"""

import math
import os
from contextlib import ExitStack

import numpy as np

B, C, S = 4, 512, 8192
E, D = 256, 256
NCORES = 8
HALF = S // 2            # tokens per core
NB = HALF // D           # 16 blocks per core
HB = D // 2              # halo = 128
TH = HALF + 2 * HB       # halo'd token range = 4352
W = 2 * D                # window width 512

_PROG_CACHE = {}
LAST_RESULT = None


def _build_program(has_bias: bool, has_mask: bool):
    import concourse.tile as tile
    from concourse import bacc, mybir

    f32 = mybir.dt.float32
    fr = mybir.dt.float32r
    AF = mybir.ActivationFunctionType

    nc = bacc.Bacc("TRN2", target_bir_lowering=False, debug=False)

    x_d = nc.dram_tensor("x_halo", [C, TH], fr, kind="ExternalInput").ap()
    wq_d = nc.dram_tensor("wq_t", [128, 4, 2, 128], fr, kind="ExternalInput").ap()
    wk_d = nc.dram_tensor("wk_t", [128, 4, 2, 128], fr, kind="ExternalInput").ap()
    wv_d = nc.dram_tensor("wv_t", [128, 4, 256], fr, kind="ExternalInput").ap()
    wo_d = nc.dram_tensor("wo_t", [128, 2, 4, 128], fr, kind="ExternalInput").ap()
    lcol_d = nc.dram_tensor("lcol", [128, NB * 4], f32, kind="ExternalInput").ap()
    fcol_d = nc.dram_tensor("fcol", [128, NB * 4], f32, kind="ExternalInput").ap()
    onec_d = nc.dram_tensor("onec", [128, 1], fr, kind="ExternalInput").ap()
    oner_d = nc.dram_tensor("oner", [1, 128], fr, kind="ExternalInput").ap()
    if has_bias:
        bq_d = nc.dram_tensor("bq2", [128, 2], f32, kind="ExternalInput").ap()
        bk_d = nc.dram_tensor("bk2", [128, 2], f32, kind="ExternalInput").ap()
        bv_d = nc.dram_tensor("bvr", [1, 256], fr, kind="ExternalInput").ap()
        bo_d = nc.dram_tensor("bo4", [128, 4], f32, kind="ExternalInput").ap()
    if has_mask:
        mr_d = nc.dram_tensor("mrow", [1, HALF], fr, kind="ExternalInput").ap()
    y_d = nc.dram_tensor("y", [C, HALF], f32, kind="ExternalOutput").ap()

    with tile.TileContext(nc) as tc, ExitStack() as ctx:
        ctx.enter_context(nc.allow_low_precision(
            reason="float32r is 4-byte fp32-rounded; matmul accum stays fp32"))
        consts = ctx.enter_context(tc.tile_pool(name="consts", bufs=1))
        qkpool = ctx.enter_context(tc.tile_pool(name="qkpool", bufs=1))
        vtpool = ctx.enter_context(tc.tile_pool(name="vtpool", bufs=1))
        avpool = ctx.enter_context(tc.tile_pool(name="avpool", bufs=1))

        wq_sb = consts.tile([128, 4, 2, 128], fr)
        nc.sync.dma_start(out=wq_sb[:], in_=wq_d[:])
        wk_sb = consts.tile([128, 4, 2, 128], fr)
        nc.sync.dma_start(out=wk_sb[:], in_=wk_d[:])
        wv_sb = consts.tile([128, 4, 256], fr)
        nc.sync.dma_start(out=wv_sb[:], in_=wv_d[:])
        wo_sb = consts.tile([128, 2, 4, 128], fr)
        nc.sync.dma_start(out=wo_sb[:], in_=wo_d[:])
        lcol_sb = consts.tile([128, NB * 4], f32)
        nc.sync.dma_start(out=lcol_sb[:], in_=lcol_d[:])
        fcol_sb = consts.tile([128, NB * 4], f32)
        nc.sync.dma_start(out=fcol_sb[:], in_=fcol_d[:])
        ones_col = consts.tile([128, 1], fr)
        nc.sync.dma_start(out=ones_col[:], in_=onec_d[:])
        ones_row = consts.tile([1, 128], fr)
        nc.sync.dma_start(out=ones_row[:], in_=oner_d[:])
        if has_bias:
            bq_sb = consts.tile([128, 2], f32)
            nc.sync.dma_start(out=bq_sb[:], in_=bq_d[:])
            bk_sb = consts.tile([128, 2], f32)
            nc.sync.dma_start(out=bk_sb[:], in_=bk_d[:])
            bv_sb = consts.tile([1, 256], fr)
            nc.sync.dma_start(out=bv_sb[:], in_=bv_d[:])
            bo_sb = consts.tile([128, 4], f32)
            nc.sync.dma_start(out=bo_sb[:], in_=bo_d[:])
        if has_mask:
            mr_sb = consts.tile([1, HALF], fr)
            nc.sync.dma_start(out=mr_sb[:], in_=mr_d[:])

        # persistent projections (cover the full halo'd range)
        qw_sb = [qkpool.tile([128, TH], fr, name=f"qw{ec}") for ec in range(2)]
        kw_sb = [qkpool.tile([128, TH], fr, name=f"kw{ec}") for ec in range(2)]
        vt_sb = vtpool.tile([128, TH // 128, 256], fr)  # [tok%128, tokchunk, e]
        avn_sb = avpool.tile([128, NB, 2, 256], fr)     # pre-gelu normalized av

        # ---------------- phase 1: projections ----------------
        tts = [(i * 512, 512) for i in range(8)] + [(4096, 256)]
        with tc.tile_pool(name="xp", bufs=2) as xp, \
             tc.tile_pool(name="pj", bufs=1, space="PSUM") as pj:
            for (t0, tw) in tts:
                x_t = [xp.tile([128, 512], fr, tag=f"x{cc}", name=f"x{cc}")
                       for cc in range(4)]
                for cc in range(4):
                    nc.sync.dma_start(
                        out=x_t[cc][:, :tw],
                        in_=x_d[cc * 128:(cc + 1) * 128, t0:t0 + tw])
                # q/k projections: [E, token] layout
                for (w_sb, b_sb, out_sb) in (
                    (wq_sb, "bq", qw_sb), (wk_sb, "bk", kw_sb)):
                    for ec in range(2):
                        ps = pj.tile([128, 512], f32, tag="qk", bufs=4, name="qkps")
                        for cc in range(4):
                            nc.tensor.matmul(
                                out=ps[:, :tw],
                                lhsT=w_sb[:, cc, ec, :],
                                rhs=x_t[cc][:, :tw],
                                start=(cc == 0), stop=(cc == 3))
                        if has_bias:
                            bias = (bq_sb if b_sb == "bq" else bk_sb)[:, ec:ec + 1]
                            nc.scalar.activation(
                                out=out_sb[ec][:, t0:t0 + tw], in_=ps[:, :tw],
                                func=AF.Identity, bias=bias)
                        else:
                            nc.scalar.copy(
                                out=out_sb[ec][:, t0:t0 + tw], in_=ps[:, :tw])
                # vT projection: [token, E] layout
                for tci in range(tw // 128):
                    g = t0 // 128 + tci
                    vp = pj.tile([128, 256], f32, tag="v", bufs=2, name="vps")
                    for cc in range(4):
                        nc.tensor.matmul(
                            out=vp[:],
                            lhsT=x_t[cc][:, tci * 128:(tci + 1) * 128],
                            rhs=wv_sb[:, cc, :],
                            start=(cc == 0), stop=(cc == 3 and not has_bias))
                    if has_bias:
                        nc.tensor.matmul(
                            out=vp[:], lhsT=ones_row[:], rhs=bv_sb[:],
                            start=False, stop=True)
                    nc.scalar.copy(out=vt_sb[:, g, :], in_=vp[:])

        # ---------------- phase 2a: attention ----------------
        with tc.tile_pool(name="sp", bufs=2) as sp, \
             tc.tile_pool(name="ap2", bufs=1, space="PSUM") as ap2:
            for n in range(NB):
                base = n * 256
                e_ps = [ap2.tile([128, 256], f32, tag="e", bufs=4, name=f"e{jc}")
                        for jc in range(4)]
                for jc in range(4):
                    for ec in range(2):
                        nc.tensor.matmul(
                            out=e_ps[jc][:],
                            lhsT=kw_sb[ec][:, base + jc * 128:base + (jc + 1) * 128],
                            rhs=qw_sb[ec][:, HB + base:HB + base + 256],
                            start=(ec == 0), stop=(ec == 1))
                s_t = sp.tile([128, 4, 256], fr, tag="s", name="s_t")
                for jc in range(4):
                    nc.scalar.activation(
                        out=s_t[:, jc, :], in_=e_ps[jc][:], func=AF.Exp,
                        bias=lcol_sb[:, n * 4 + jc:n * 4 + jc + 1],
                        scale=1.0 / math.sqrt(E))
                z_ps = ap2.tile([1, 256], f32, tag="zz", bufs=2, name="z_ps")
                for jc in range(4):
                    nc.tensor.matmul(
                        out=z_ps[:], lhsT=ones_col[:], rhs=s_t[:, jc, :],
                        start=(jc == 0), stop=(jc == 3))
                zinv = sp.tile([1, 256], fr, tag="zinv", name="zinv")
                nc.vector.reciprocal(out=zinv[:], in_=z_ps[:])
                zb_ps = ap2.tile([128, 256], f32, tag="zz", bufs=2, name="zb_ps")
                nc.tensor.matmul(out=zb_ps[:], lhsT=ones_row[:], rhs=zinv[:],
                                 start=True, stop=True)
                zb_sb = sp.tile([128, 256], f32, tag="zb", name="zb_sb")
                nc.scalar.copy(out=zb_sb[:], in_=zb_ps[:])
                s2_t = sp.tile([128, 4, 256], fr, tag="s2", name="s2_t")
                for jc in range(4):
                    nc.vector.tensor_scalar_mul(
                        s2_t[:, jc, :], s_t[:, jc, :],
                        fcol_sb[:, n * 4 + jc:n * 4 + jc + 1])
                av_ps = [ap2.tile([128, 256], f32, tag="av", bufs=2, name=f"av{ec}")
                         for ec in range(2)]
                for ec in range(2):
                    for jc in range(4):
                        nc.tensor.matmul(
                            out=av_ps[ec][:],
                            lhsT=vt_sb[:, 2 * n + jc, ec * 128:(ec + 1) * 128],
                            rhs=s2_t[:, jc, :],
                            start=(jc == 0), stop=(jc == 3))
                for ec in range(2):
                    nc.vector.tensor_mul(
                        avn_sb[:, n, ec, :], av_ps[ec][:], zb_sb[:])

        # ---------------- phase 2b: gelu (one ACT table set) ----------------
        for n in range(NB):
            for ec in range(2):
                nc.scalar.activation(
                    out=avn_sb[:, n, ec, :], in_=avn_sb[:, n, ec, :],
                    func=AF.Gelu)

        # ---------------- phase 2c: output projection ----------------
        with tc.tile_pool(name="yp", bufs=2) as yp, \
             tc.tile_pool(name="ap3", bufs=1, space="PSUM") as ap3:
            for n in range(NB):
                y_ps = [ap3.tile([128, 256], f32, tag="y", bufs=4, name=f"y{cc}")
                        for cc in range(4)]
                for cc in range(4):
                    for ec in range(2):
                        nc.tensor.matmul(
                            out=y_ps[cc][:],
                            lhsT=wo_sb[:, ec, cc, :],
                            rhs=avn_sb[:, n, ec, :],
                            start=(ec == 0), stop=(ec == 1))
                if has_mask:
                    mb_ps = ap3.tile([128, 256], f32, tag="mb", bufs=2, name="mb")
                    nc.tensor.matmul(
                        out=mb_ps[:], lhsT=ones_row[:],
                        rhs=mr_sb[:, n * 256:(n + 1) * 256],
                        start=True, stop=True)
                y_sb = yp.tile([128, 4, 256], f32, tag="ysb", name="y_sb")
                for cc in range(4):
                    if has_bias:
                        nc.scalar.activation(
                            out=y_sb[:, cc, :], in_=y_ps[cc][:],
                            func=AF.Identity, bias=bo_sb[:, cc:cc + 1])
                    else:
                        nc.scalar.copy(out=y_sb[:, cc, :], in_=y_ps[cc][:])
                    if has_mask:
                        nc.vector.tensor_mul(
                            y_sb[:, cc, :], y_sb[:, cc, :], mb_ps[:])
                    nc.sync.dma_start(
                        out=y_d[cc * 128:(cc + 1) * 128, n * 256:(n + 1) * 256],
                        in_=y_sb[:, cc, :])

    nc.compile()
    return nc


def get_program(has_bias: bool, has_mask: bool):
    key = (has_bias, has_mask)
    if key not in _PROG_CACHE:
        _PROG_CACHE[key] = _build_program(has_bias, has_mask)
    return _PROG_CACHE[key]


def _host_prep(x1, mask, Wq, bq, Wk, bk, Wv, bv, Wo, bo, has_bias, has_mask):
    """Build the per-core input maps (sharding + layout only)."""
    wq_t = np.ascontiguousarray(
        Wq.reshape(2, 128, 4, 128).transpose(3, 2, 0, 1))   # [p, cc, ec, f]
    wk_t = np.ascontiguousarray(
        Wk.reshape(2, 128, 4, 128).transpose(3, 2, 0, 1))
    wv_t = np.ascontiguousarray(
        Wv.reshape(256, 4, 128).transpose(2, 1, 0))          # [p, cc, e]
    wo_t = np.ascontiguousarray(
        Wo.reshape(4, 128, 2, 128).transpose(3, 2, 0, 1))    # [p, ec, cc, f]

    win = (np.arange(W) < W - 1).astype(np.float32)          # [512]
    in_maps = []
    for b in range(B):
        xp = np.pad(x1[b], ((0, 0), (HB, HB)))               # [C, S + 2HB]
        pmp = np.pad(mask[b, 0], (HB, HB))                   # [S + 2HB]
        for h in range(2):
            start = h * HALF
            x_halo = np.ascontiguousarray(xp[:, start:start + TH])
            # mask columns per local block
            lcol = np.empty((128, NB * 4), np.float32)
            fcol = np.empty((128, NB * 4), np.float32)
            for n in range(NB):
                gtok = start + n * D                         # padded-idx base
                pw = pmp[gtok:gtok + W].astype(np.float32)   # [512]
                f = (win * pw).astype(np.float32)
                lf = np.log(f + np.float32(1e-6)).astype(np.float32)
                fcol[:, n * 4:(n + 1) * 4] = f.reshape(4, 128).T
                lcol[:, n * 4:(n + 1) * 4] = lf.reshape(4, 128).T
            im = {
                "x_halo": x_halo, "wq_t": wq_t, "wk_t": wk_t,
                "wv_t": wv_t, "wo_t": wo_t, "lcol": lcol, "fcol": fcol,
                "onec": np.ones((128, 1), np.float32),
                "oner": np.ones((1, 128), np.float32),
            }
            if has_bias:
                im["bq2"] = np.ascontiguousarray(bq.reshape(2, 128).T)
                im["bk2"] = np.ascontiguousarray(bk.reshape(2, 128).T)
                im["bvr"] = np.ascontiguousarray(bv.reshape(1, 256))
                im["bo4"] = np.ascontiguousarray(bo.reshape(4, 128).T)
            if has_mask:
                im["mrow"] = np.ascontiguousarray(
                    mask[b, 0, start:start + HALF].reshape(1, HALF))
            in_maps.append(im)
    return in_maps


def kernel(x1, mask, Wq, bq, Wk, bk, Wv, bv, Wo, bo):
    global LAST_RESULT
    from concourse.bass_utils import run_bass_kernel_spmd

    x1 = np.asarray(x1, np.float32)
    mask = np.asarray(mask, np.float32)
    Wq, bq = np.asarray(Wq, np.float32), np.asarray(bq, np.float32)
    Wk, bk = np.asarray(Wk, np.float32), np.asarray(bk, np.float32)
    Wv, bv = np.asarray(Wv, np.float32), np.asarray(bv, np.float32)
    Wo, bo = np.asarray(Wo, np.float32), np.asarray(bo, np.float32)

    has_bias = bool(np.any(bq) or np.any(bk) or np.any(bv) or np.any(bo))
    has_mask = not bool(np.all(mask == 1.0))

    nc = get_program(has_bias, has_mask)
    in_maps = _host_prep(x1, mask, Wq, bq, Wk, bk, Wv, bv, Wo, bo,
                         has_bias, has_mask)
    res = run_bass_kernel_spmd(nc, in_maps, core_ids=list(range(NCORES)))
    LAST_RESULT = res

    y = np.empty((B, C, S), np.float32)
    for b in range(B):
        for h in range(2):
            y[b, :, h * HALF:(h + 1) * HALF] = res.results[b * 2 + h]["y"]
    return y


# revision 13
# speedup vs baseline: 1.2660x; 1.2660x over previous
"""Trainium2 Bass kernel for windowed (block-sparse) attention encoder.

Model (reference):
  q/k/v = 1x1 conv projections of x1 [B,C,S] with weights [E,C]
  queries split into nb = S/D blocks of D tokens; k/v use overlapping
  windows of width 2D (stride D, halo D/2 each side, zero-padded)
  attn = softmax(qk/sqrt(E) + log(fmask+1e-6)) * fmask
  y = Wo @ gelu(attn @ v) + bo, masked by the padding mask.

Sharding: 8 cores = batch (4) x sequence halves (2). Each core gets a
halo'd x slice [C, S/2 + D] so no cross-core communication is needed.

Device layout (per core):
  qw/kw: [E(part), token]   energyT[j,q] = kw^T qw  (j on partitions)
  vT:    [token(part), E]   av[e,q] = vT^T s2
  softmax runs along the partition (j) dim with no max subtraction
  (energies are O(+-10), exp is fp32-safe); the window/padding mask
  enters as a per-partition log-bias on the exp and a per-partition
  multiplier, so no partition-broadcasts are needed anywhere.
All matmul operands use float32r (fast fp32 path on the PE).
"""

import math
import os
from contextlib import ExitStack

import numpy as np

B, C, S = 4, 512, 8192
E, D = 256, 256
NCORES = 8
HALF = S // 2            # tokens per core
NB = HALF // D           # 16 blocks per core
HB = D // 2              # halo = 128
TH = HALF + 2 * HB       # halo'd token range = 4352
W = 2 * D                # window width 512

_PROG_CACHE = {}
LAST_RESULT = None


def _build_program(has_bias: bool, has_mask: bool):
    import concourse.tile as tile
    from concourse import bacc, mybir

    f32 = mybir.dt.float32
    fr = mybir.dt.float32r
    AF = mybir.ActivationFunctionType

    nc = bacc.Bacc("TRN2", target_bir_lowering=False, debug=False)

    x_d = nc.dram_tensor("x_halo", [C, TH], fr, kind="ExternalInput").ap()
    wq_d = nc.dram_tensor("wq_t", [128, 4, 2, 128], fr, kind="ExternalInput").ap()
    wk_d = nc.dram_tensor("wk_t", [128, 4, 2, 128], fr, kind="ExternalInput").ap()
    wv_d = nc.dram_tensor("wv_t", [128, 4, 256], fr, kind="ExternalInput").ap()
    wo_d = nc.dram_tensor("wo_t", [128, 2, 4, 128], fr, kind="ExternalInput").ap()
    lcol_d = nc.dram_tensor("lcol", [128, NB * 4], f32, kind="ExternalInput").ap()
    fcol_d = nc.dram_tensor("fcol", [128, NB * 4], f32, kind="ExternalInput").ap()
    onem_d = nc.dram_tensor("onem", [128, 128], fr, kind="ExternalInput").ap()
    oner_d = nc.dram_tensor("oner", [1, 128], fr, kind="ExternalInput").ap()
    if has_bias:
        bq_d = nc.dram_tensor("bq2", [128, 2], f32, kind="ExternalInput").ap()
        bk_d = nc.dram_tensor("bk2", [128, 2], f32, kind="ExternalInput").ap()
        bv_d = nc.dram_tensor("bvr", [1, 256], fr, kind="ExternalInput").ap()
        bo_d = nc.dram_tensor("bo4", [128, 4], f32, kind="ExternalInput").ap()
    if has_mask:
        mr_d = nc.dram_tensor("mrow", [1, HALF], fr, kind="ExternalInput").ap()
    y_d = nc.dram_tensor("y", [C, HALF], f32, kind="ExternalOutput").ap()

    with tile.TileContext(nc) as tc, ExitStack() as ctx:
        ctx.enter_context(nc.allow_low_precision(
            reason="float32r is 4-byte fp32-rounded; matmul accum stays fp32"))
        consts = ctx.enter_context(tc.tile_pool(name="consts", bufs=1))
        qkpool = ctx.enter_context(tc.tile_pool(name="qkpool", bufs=1))
        vtpool = ctx.enter_context(tc.tile_pool(name="vtpool", bufs=1))
        avpool = ctx.enter_context(tc.tile_pool(name="avpool", bufs=1))

        wq_sb = consts.tile([128, 4, 2, 128], fr)
        nc.sync.dma_start(out=wq_sb[:], in_=wq_d[:])
        wk_sb = consts.tile([128, 4, 2, 128], fr)
        nc.sync.dma_start(out=wk_sb[:], in_=wk_d[:])
        wv_sb = consts.tile([128, 4, 256], fr)
        nc.sync.dma_start(out=wv_sb[:], in_=wv_d[:])
        wo_sb = consts.tile([128, 2, 4, 128], fr)
        nc.sync.dma_start(out=wo_sb[:], in_=wo_d[:])
        lcol_sb = consts.tile([128, NB * 4], f32)
        nc.sync.dma_start(out=lcol_sb[:], in_=lcol_d[:])
        fcol_sb = consts.tile([128, NB * 4], f32)
        nc.sync.dma_start(out=fcol_sb[:], in_=fcol_d[:])
        ones_mat = consts.tile([128, 128], fr)
        nc.sync.dma_start(out=ones_mat[:], in_=onem_d[:])
        ones_row = consts.tile([1, 128], fr)
        nc.sync.dma_start(out=ones_row[:], in_=oner_d[:])
        if has_bias:
            bq_sb = consts.tile([128, 2], f32)
            nc.sync.dma_start(out=bq_sb[:], in_=bq_d[:])
            bk_sb = consts.tile([128, 2], f32)
            nc.sync.dma_start(out=bk_sb[:], in_=bk_d[:])
            bv_sb = consts.tile([1, 256], fr)
            nc.sync.dma_start(out=bv_sb[:], in_=bv_d[:])
            bo_sb = consts.tile([128, 4], f32)
            nc.sync.dma_start(out=bo_sb[:], in_=bo_d[:])
        if has_mask:
            mr_sb = consts.tile([1, HALF], fr)
            nc.sync.dma_start(out=mr_sb[:], in_=mr_d[:])

        # persistent projections (cover the full halo'd range)
        qw_sb = [qkpool.tile([128, TH], fr, name=f"qw{ec}") for ec in range(2)]
        kw_sb = [qkpool.tile([128, TH], fr, name=f"kw{ec}") for ec in range(2)]
        vt_sb = vtpool.tile([128, TH // 128, 256], fr)  # [tok%128, tokchunk, e]
        avn_sb = avpool.tile([128, 2, NB * 256], fr)    # pre-gelu normalized av

        # ---------------- phase 1: projections ----------------
        tts = [(i * 512, 512) for i in range(8)] + [(4096, 256)]
        with tc.tile_pool(name="xp", bufs=2) as xp, \
             tc.tile_pool(name="pj", bufs=1, space="PSUM") as pj:
            for (t0, tw) in tts:
                x_t = [xp.tile([128, 512], fr, tag=f"x{cc}", name=f"x{cc}")
                       for cc in range(4)]
                for cc in range(4):
                    nc.sync.dma_start(
                        out=x_t[cc][:, :tw],
                        in_=x_d[cc * 128:(cc + 1) * 128, t0:t0 + tw])
                # q/k projections: [E, token] layout
                for (w_sb, b_sb, out_sb) in (
                    (wq_sb, "bq", qw_sb), (wk_sb, "bk", kw_sb)):
                    for ec in range(2):
                        ps = pj.tile([128, 512], f32, tag="qk", bufs=4, name="qkps")
                        for cc in range(4):
                            nc.tensor.matmul(
                                out=ps[:, :tw],
                                lhsT=w_sb[:, cc, ec, :],
                                rhs=x_t[cc][:, :tw],
                                start=(cc == 0), stop=(cc == 3))
                        if has_bias:
                            bias = (bq_sb if b_sb == "bq" else bk_sb)[:, ec:ec + 1]
                            nc.scalar.activation(
                                out=out_sb[ec][:, t0:t0 + tw], in_=ps[:, :tw],
                                func=AF.Identity, bias=bias)
                        else:
                            nc.scalar.copy(
                                out=out_sb[ec][:, t0:t0 + tw], in_=ps[:, :tw])
                # vT projection: [token, E] layout
                for tci in range(tw // 128):
                    g = t0 // 128 + tci
                    vp = pj.tile([128, 256], f32, tag="v", bufs=2, name="vps")
                    for cc in range(4):
                        nc.tensor.matmul(
                            out=vp[:],
                            lhsT=x_t[cc][:, tci * 128:(tci + 1) * 128],
                            rhs=wv_sb[:, cc, :],
                            start=(cc == 0), stop=(cc == 3 and not has_bias))
                    if has_bias:
                        nc.tensor.matmul(
                            out=vp[:], lhsT=ones_row[:], rhs=bv_sb[:],
                            start=False, stop=True)
                    nc.vector.tensor_copy(vt_sb[:, g, :], vp[:])

        # ---------------- phase 2a: attention ----------------
        with tc.tile_pool(name="sp", bufs=2) as sp, \
             tc.tile_pool(name="ap2", bufs=1, space="PSUM") as ap2:
            for n in range(NB):
                base = n * 256
                e_ps = [ap2.tile([128, 256], f32, tag="e", bufs=4, name=f"e{jc}")
                        for jc in range(4)]
                for jc in range(4):
                    for ec in range(2):
                        nc.tensor.matmul(
                            out=e_ps[jc][:],
                            lhsT=kw_sb[ec][:, base + jc * 128:base + (jc + 1) * 128],
                            rhs=qw_sb[ec][:, HB + base:HB + base + 256],
                            start=(ec == 0), stop=(ec == 1))
                s_t = sp.tile([128, 4, 256], fr, tag="s", name="s_t")
                for jc in range(4):
                    nc.scalar.activation(
                        out=s_t[:, jc, :], in_=e_ps[jc][:], func=AF.Exp,
                        bias=lcol_sb[:, n * 4 + jc:n * 4 + jc + 1],
                        scale=1.0 / math.sqrt(E))
                # zb[p, q] = sum_j s~[j, q]  (Z broadcast across partitions)
                zb_ps = ap2.tile([128, 256], f32, tag="zz", bufs=2, name="zb_ps")
                for jc in range(4):
                    nc.tensor.matmul(
                        out=zb_ps[:], lhsT=ones_mat[:], rhs=s_t[:, jc, :],
                        start=(jc == 0), stop=(jc == 3))
                zrec = sp.tile([128, 256], f32, tag="zrec", name="zrec")
                nc.vector.reciprocal(out=zrec[:], in_=zb_ps[:])
                s2_t = sp.tile([128, 4, 256], fr, tag="s2", name="s2_t")
                for jc in range(4):
                    nc.vector.tensor_scalar_mul(
                        s2_t[:, jc, :], s_t[:, jc, :],
                        fcol_sb[:, n * 4 + jc:n * 4 + jc + 1])
                av_ps = [ap2.tile([128, 256], f32, tag="av", bufs=2, name=f"av{ec}")
                         for ec in range(2)]
                for ec in range(2):
                    for jc in range(4):
                        nc.tensor.matmul(
                            out=av_ps[ec][:],
                            lhsT=vt_sb[:, 2 * n + jc, ec * 128:(ec + 1) * 128],
                            rhs=s2_t[:, jc, :],
                            start=(jc == 0), stop=(jc == 3))
                for ec in range(2):
                    nc.vector.tensor_mul(
                        avn_sb[:, ec, n * 256:(n + 1) * 256],
                        av_ps[ec][:], zrec[:])

        # ---- phase 2b: gelu, fenced off so the ACT table set loads once ----
        tc.no_sync_barrier()
        for p in range(NB // 2):
            for ec in range(2):
                nc.scalar.activation(
                    out=avn_sb[:, ec, p * 512:(p + 1) * 512],
                    in_=avn_sb[:, ec, p * 512:(p + 1) * 512],
                    func=AF.Gelu)

        # ------- phase 2c: output projection (block pairs, N=512) -------
        with tc.tile_pool(name="yp", bufs=2) as yp, \
             tc.tile_pool(name="ap3", bufs=1, space="PSUM") as ap3:
            for p in range(NB // 2):
                y_ps = [ap3.tile([128, 512], f32, tag="y", bufs=4, name=f"y{cc}")
                        for cc in range(4)]
                for cc in range(4):
                    for ec in range(2):
                        nc.tensor.matmul(
                            out=y_ps[cc][:],
                            lhsT=wo_sb[:, ec, cc, :],
                            rhs=avn_sb[:, ec, p * 512:(p + 1) * 512],
                            start=(ec == 0), stop=(ec == 1))
                if has_mask:
                    mb_ps = ap3.tile([128, 512], f32, tag="mb", bufs=2, name="mb")
                    nc.tensor.matmul(
                        out=mb_ps[:], lhsT=ones_row[:],
                        rhs=mr_sb[:, p * 512:(p + 1) * 512],
                        start=True, stop=True)
                y_sb = yp.tile([128, 4, 512], f32, tag="ysb", name="y_sb")
                for cc in range(4):
                    if has_bias:
                        nc.scalar.activation(
                            out=y_sb[:, cc, :], in_=y_ps[cc][:],
                            func=AF.Identity, bias=bo_sb[:, cc:cc + 1])
                    else:
                        nc.vector.tensor_copy(y_sb[:, cc, :], y_ps[cc][:])
                    if has_mask:
                        nc.vector.tensor_mul(
                            y_sb[:, cc, :], y_sb[:, cc, :], mb_ps[:])
                    nc.sync.dma_start(
                        out=y_d[cc * 128:(cc + 1) * 128, p * 512:(p + 1) * 512],
                        in_=y_sb[:, cc, :])

    nc.compile()
    return nc


def get_program(has_bias: bool, has_mask: bool):
    key = (has_bias, has_mask)
    if key not in _PROG_CACHE:
        _PROG_CACHE[key] = _build_program(has_bias, has_mask)
    return _PROG_CACHE[key]


def _host_prep(x1, mask, Wq, bq, Wk, bk, Wv, bv, Wo, bo, has_bias, has_mask):
    """Build the per-core input maps (sharding + layout only)."""
    wq_t = np.ascontiguousarray(
        Wq.reshape(2, 128, 4, 128).transpose(3, 2, 0, 1))   # [p, cc, ec, f]
    wk_t = np.ascontiguousarray(
        Wk.reshape(2, 128, 4, 128).transpose(3, 2, 0, 1))
    wv_t = np.ascontiguousarray(
        Wv.reshape(256, 4, 128).transpose(2, 1, 0))          # [p, cc, e]
    wo_t = np.ascontiguousarray(
        Wo.reshape(4, 128, 2, 128).transpose(3, 2, 0, 1))    # [p, ec, cc, f]

    win = (np.arange(W) < W - 1).astype(np.float32)          # [512]
    in_maps = []
    for b in range(B):
        xp = np.pad(x1[b], ((0, 0), (HB, HB)))               # [C, S + 2HB]
        pmp = np.pad(mask[b, 0], (HB, HB))                   # [S + 2HB]
        for h in range(2):
            start = h * HALF
            x_halo = np.ascontiguousarray(xp[:, start:start + TH])
            # mask columns per local block
            lcol = np.empty((128, NB * 4), np.float32)
            fcol = np.empty((128, NB * 4), np.float32)
            for n in range(NB):
                gtok = start + n * D                         # padded-idx base
                pw = pmp[gtok:gtok + W].astype(np.float32)   # [512]
                f = (win * pw).astype(np.float32)
                lf = np.log(f + np.float32(1e-6)).astype(np.float32)
                fcol[:, n * 4:(n + 1) * 4] = f.reshape(4, 128).T
                lcol[:, n * 4:(n + 1) * 4] = lf.reshape(4, 128).T
            im = {
                "x_halo": x_halo, "wq_t": wq_t, "wk_t": wk_t,
                "wv_t": wv_t, "wo_t": wo_t, "lcol": lcol, "fcol": fcol,
                "onem": np.ones((128, 128), np.float32),
                "oner": np.ones((1, 128), np.float32),
            }
            if has_bias:
                im["bq2"] = np.ascontiguousarray(bq.reshape(2, 128).T)
                im["bk2"] = np.ascontiguousarray(bk.reshape(2, 128).T)
                im["bvr"] = np.ascontiguousarray(bv.reshape(1, 256))
                im["bo4"] = np.ascontiguousarray(bo.reshape(4, 128).T)
            if has_mask:
                im["mrow"] = np.ascontiguousarray(
                    mask[b, 0, start:start + HALF].reshape(1, HALF))
            in_maps.append(im)
    return in_maps


def kernel(x1, mask, Wq, bq, Wk, bk, Wv, bv, Wo, bo):
    global LAST_RESULT
    from concourse.bass_utils import run_bass_kernel_spmd

    x1 = np.asarray(x1, np.float32)
    mask = np.asarray(mask, np.float32)
    Wq, bq = np.asarray(Wq, np.float32), np.asarray(bq, np.float32)
    Wk, bk = np.asarray(Wk, np.float32), np.asarray(bk, np.float32)
    Wv, bv = np.asarray(Wv, np.float32), np.asarray(bv, np.float32)
    Wo, bo = np.asarray(Wo, np.float32), np.asarray(bo, np.float32)

    has_bias = bool(np.any(bq) or np.any(bk) or np.any(bv) or np.any(bo))
    has_mask = not bool(np.all(mask == 1.0))

    nc = get_program(has_bias, has_mask)
    in_maps = _host_prep(x1, mask, Wq, bq, Wk, bk, Wv, bv, Wo, bo,
                         has_bias, has_mask)
    res = run_bass_kernel_spmd(nc, in_maps, core_ids=list(range(NCORES)))
    LAST_RESULT = res

    y = np.empty((B, C, S), np.float32)
    for b in range(B):
        for h in range(2):
            y[b, :, h * HALF:(h + 1) * HALF] = res.results[b * 2 + h]["y"]
    return y


# revision 15
# speedup vs baseline: 1.4315x; 1.1307x over previous
"""Trainium2 Bass kernel for windowed (block-sparse) attention encoder.

Model (reference):
  q/k/v = 1x1 conv projections of x1 [B,C,S] with weights [E,C]
  queries split into nb = S/D blocks of D tokens; k/v use overlapping
  windows of width 2D (stride D, halo D/2 each side, zero-padded)
  attn = softmax(qk/sqrt(E) + log(fmask+1e-6)) * fmask
  y = Wo @ gelu(attn @ v) + bo, masked by the padding mask.

Sharding: 8 cores = batch (4) x sequence halves (2). Each core gets a
halo'd x slice [C, S/2 + D] so no cross-core communication is needed.

Device layout (per core):
  qw/kw: [E(part), token]   energyT[j,q] = kw^T qw  (j on partitions)
  vT:    [token(part), E]   av[e,q] = vT^T s2
  softmax runs along the partition (j) dim with no max subtraction
  (energies are O(+-10), exp is fp32-safe); the window/padding mask
  enters as a per-partition log-bias on the exp and a per-partition
  multiplier, so no partition-broadcasts are needed anywhere.
All matmul operands use float32r (fast fp32 path on the PE).
Projections and attention are interleaved per token tile so the PE
never drains between phases; gelu is fenced into its own region so the
ACT function-table set loads exactly once.
"""

import math
import os
from contextlib import ExitStack

import numpy as np

B, C, S = 4, 512, 8192
E, D = 256, 256
NCORES = 8
HALF = S // 2            # tokens per core
NB = HALF // D           # 16 blocks per core
HB = D // 2              # halo = 128
TH = HALF + 2 * HB       # halo'd token range = 4352
W = 2 * D                # window width 512

_PROG_CACHE = {}
LAST_RESULT = None


def _build_program(has_bias: bool, has_mask: bool):
    import concourse.tile as tile
    from concourse import bacc, mybir

    f32 = mybir.dt.float32
    fr = mybir.dt.float32r
    AF = mybir.ActivationFunctionType

    nc = bacc.Bacc("TRN2", target_bir_lowering=False, debug=False)

    x_d = nc.dram_tensor("x_halo", [C, TH], fr, kind="ExternalInput").ap()
    wq_d = nc.dram_tensor("wq_t", [128, 4, 2, 128], fr, kind="ExternalInput").ap()
    wk_d = nc.dram_tensor("wk_t", [128, 4, 2, 128], fr, kind="ExternalInput").ap()
    wv_d = nc.dram_tensor("wv_t", [128, 4, 256], fr, kind="ExternalInput").ap()
    wo_d = nc.dram_tensor("wo_t", [128, 2, 4, 128], fr, kind="ExternalInput").ap()
    lcol_d = nc.dram_tensor("lcol", [128, NB * 4], f32, kind="ExternalInput").ap()
    fcol_d = nc.dram_tensor("fcol", [128, NB * 4], f32, kind="ExternalInput").ap()
    onem_d = nc.dram_tensor("onem", [128, 128], fr, kind="ExternalInput").ap()
    if has_bias or has_mask:
        oner_d = nc.dram_tensor("oner", [1, 128], fr, kind="ExternalInput").ap()
    if has_bias:
        bq_d = nc.dram_tensor("bq2", [128, 2], f32, kind="ExternalInput").ap()
        bk_d = nc.dram_tensor("bk2", [128, 2], f32, kind="ExternalInput").ap()
        bv_d = nc.dram_tensor("bvr", [1, 256], fr, kind="ExternalInput").ap()
        bo_d = nc.dram_tensor("bo4", [128, 4], f32, kind="ExternalInput").ap()
    if has_mask:
        mr_d = nc.dram_tensor("mrow", [1, HALF], fr, kind="ExternalInput").ap()
    y_d = nc.dram_tensor("y", [C, HALF], f32, kind="ExternalOutput").ap()

    with tile.TileContext(nc) as tc, ExitStack() as ctx:
        ctx.enter_context(nc.allow_low_precision(
            reason="float32r is 4-byte fp32-rounded; matmul accum stays fp32"))
        consts = ctx.enter_context(tc.tile_pool(name="consts", bufs=1))
        qkpool = ctx.enter_context(tc.tile_pool(name="qkpool", bufs=1))
        vtpool = ctx.enter_context(tc.tile_pool(name="vtpool", bufs=1))
        avpool = ctx.enter_context(tc.tile_pool(name="avpool", bufs=1))

        wq_sb = consts.tile([128, 4, 2, 128], fr)
        nc.scalar.dma_start(out=wq_sb[:], in_=wq_d[:])
        wk_sb = consts.tile([128, 4, 2, 128], fr)
        nc.scalar.dma_start(out=wk_sb[:], in_=wk_d[:])
        wv_sb = consts.tile([128, 4, 256], fr)
        nc.scalar.dma_start(out=wv_sb[:], in_=wv_d[:])
        wo_sb = consts.tile([128, 2, 4, 128], fr)
        nc.scalar.dma_start(out=wo_sb[:], in_=wo_d[:])
        lcol_sb = consts.tile([128, NB * 4], f32)
        nc.scalar.dma_start(out=lcol_sb[:], in_=lcol_d[:])
        fcol_sb = consts.tile([128, NB * 4], f32)
        nc.scalar.dma_start(out=fcol_sb[:], in_=fcol_d[:])
        ones_mat = consts.tile([128, 128], fr)
        nc.scalar.dma_start(out=ones_mat[:], in_=onem_d[:])
        if has_bias or has_mask:
            ones_row = consts.tile([1, 128], fr)
            nc.scalar.dma_start(out=ones_row[:], in_=oner_d[:])
        if has_bias:
            bq_sb = consts.tile([128, 2], f32)
            nc.scalar.dma_start(out=bq_sb[:], in_=bq_d[:])
            bk_sb = consts.tile([128, 2], f32)
            nc.scalar.dma_start(out=bk_sb[:], in_=bk_d[:])
            bv_sb = consts.tile([1, 256], fr)
            nc.scalar.dma_start(out=bv_sb[:], in_=bv_d[:])
            bo_sb = consts.tile([128, 4], f32)
            nc.scalar.dma_start(out=bo_sb[:], in_=bo_d[:])
        if has_mask:
            mr_sb = consts.tile([1, HALF], fr)
            nc.scalar.dma_start(out=mr_sb[:], in_=mr_d[:])

        # persistent projections (cover the full halo'd range)
        qw_sb = [qkpool.tile([128, TH], fr, name=f"qw{ec}") for ec in range(2)]
        kw_sb = [qkpool.tile([128, TH], fr, name=f"kw{ec}") for ec in range(2)]
        vt_sb = vtpool.tile([128, TH // 128, 256], fr)  # [tok%128, tokchunk, e]
        avn_sb = avpool.tile([128, 2, NB * 256], fr)    # pre-gelu normalized av

        def emit_attention(n, sp, ps):
            base = n * 256
            e_ps = [ps.tile([128, 256], f32, tag="e", bufs=2, name=f"e{jc}")
                    for jc in range(4)]
            for jc in range(4):
                for ec in range(2):
                    nc.tensor.matmul(
                        out=e_ps[jc][:],
                        lhsT=kw_sb[ec][:, base + jc * 128:base + (jc + 1) * 128],
                        rhs=qw_sb[ec][:, HB + base:HB + base + 256],
                        start=(ec == 0), stop=(ec == 1))
            s_t = sp.tile([128, 4, 256], fr, tag="s", name="s_t")
            for jc in range(4):
                nc.scalar.activation(
                    out=s_t[:, jc, :], in_=e_ps[jc][:], func=AF.Exp,
                    bias=lcol_sb[:, n * 4 + jc:n * 4 + jc + 1],
                    scale=1.0 / math.sqrt(E))
            # zb[p, q] = sum_j s~[j, q]: pairwise DVE partial sums, then
            # one ones^T matmul to reduce across the remaining partition dim
            sp2 = sp.tile([128, 2, 256], fr, tag="sp2", bufs=1, name="sp2")
            nc.vector.tensor_add(sp2[:], s_t[:, 0:2, :], s_t[:, 2:4, :])
            ssum = sp.tile([128, 256], fr, tag="ssum", bufs=1, name="ssum")
            nc.vector.tensor_add(ssum[:], sp2[:, 0, :], sp2[:, 1, :])
            zb_ps = ps.tile([128, 256], f32, tag="zz", bufs=1, name="zb_ps")
            nc.tensor.matmul(out=zb_ps[:], lhsT=ones_mat[:], rhs=ssum[:],
                             start=True, stop=True)
            zscr = sp.tile([128, 256], f32, tag="zscr", bufs=1, name="zscr")
            zrec = sp.tile([128, 256], f32, tag="zrec", bufs=1, name="zrec")
            nc.vector.reciprocal_approx_accurate(
                out=zrec[:], in_=zb_ps[:], scratch=zscr[:])
            s2_t = sp.tile([128, 4, 256], fr, tag="s2", bufs=1, name="s2_t")
            for jc in range(4):
                nc.vector.tensor_scalar_mul(
                    s2_t[:, jc, :], s_t[:, jc, :],
                    fcol_sb[:, n * 4 + jc:n * 4 + jc + 1])
            av_ps = [ps.tile([128, 256], f32, tag="av", bufs=2, name=f"av{ec}")
                     for ec in range(2)]
            for ec in range(2):
                for jc in range(4):
                    nc.tensor.matmul(
                        out=av_ps[ec][:],
                        lhsT=vt_sb[:, 2 * n + jc, ec * 128:(ec + 1) * 128],
                        rhs=s2_t[:, jc, :],
                        start=(jc == 0), stop=(jc == 3))
            for ec in range(2):
                nc.vector.tensor_mul(
                    avn_sb[:, ec, n * 256:(n + 1) * 256],
                    av_ps[ec][:], zrec[:])

        # ---- interleaved projections + attention ----
        tts = [(i * 512, 512) for i in range(8)] + [(4096, 256)]
        next_blk = 0
        with tc.tile_pool(name="xp", bufs=2) as xp, \
             tc.tile_pool(name="sp", bufs=2) as sp, \
             tc.tile_pool(name="ps", bufs=1, space="PSUM") as ps:
            for (t0, tw) in tts:
                x_t = [xp.tile([128, 512], fr, tag=f"x{cc}", name=f"x{cc}")
                       for cc in range(4)]
                for cc in range(4):
                    eng = nc.sync if cc < 2 else nc.gpsimd
                    eng.dma_start(
                        out=x_t[cc][:, :tw],
                        in_=x_d[cc * 128:(cc + 1) * 128, t0:t0 + tw])
                # q/k projections: [E, token] layout
                for (w_sb, b_sb, out_sb) in (
                    (wq_sb, "bq", qw_sb), (wk_sb, "bk", kw_sb)):
                    for ec in range(2):
                        qk_ps = ps.tile([128, 512], f32, tag="qk", bufs=2,
                                        name="qk_ps")
                        for cc in range(4):
                            nc.tensor.matmul(
                                out=qk_ps[:, :tw],
                                lhsT=w_sb[:, cc, ec, :],
                                rhs=x_t[cc][:, :tw],
                                start=(cc == 0), stop=(cc == 3))
                        if has_bias:
                            bias = (bq_sb if b_sb == "bq" else bk_sb)[:, ec:ec + 1]
                            nc.scalar.activation(
                                out=out_sb[ec][:, t0:t0 + tw], in_=qk_ps[:, :tw],
                                func=AF.Identity, bias=bias)
                        else:
                            nc.scalar.copy(
                                out=out_sb[ec][:, t0:t0 + tw], in_=qk_ps[:, :tw])
                # vT projection: [token, E] layout
                for tci in range(tw // 128):
                    g = t0 // 128 + tci
                    vp = ps.tile([128, 256], f32, tag="v", bufs=1, name="vps")
                    for cc in range(4):
                        nc.tensor.matmul(
                            out=vp[:],
                            lhsT=x_t[cc][:, tci * 128:(tci + 1) * 128],
                            rhs=wv_sb[:, cc, :],
                            start=(cc == 0), stop=(cc == 3 and not has_bias))
                    if has_bias:
                        nc.tensor.matmul(
                            out=vp[:], lhsT=ones_row[:], rhs=bv_sb[:],
                            start=False, stop=True)
                    nc.vector.tensor_copy(vt_sb[:, g, :], vp[:])
                # attention for every block whose window is now projected
                while next_blk < NB and next_blk * 256 + 512 <= t0 + tw:
                    emit_attention(next_blk, sp, ps)
                    next_blk += 1

        # ---- gelu, fenced off so the ACT table set loads once ----
        tc.no_sync_barrier()
        for p in range(NB // 2):
            for ec in range(2):
                nc.scalar.activation(
                    out=avn_sb[:, ec, p * 512:(p + 1) * 512],
                    in_=avn_sb[:, ec, p * 512:(p + 1) * 512],
                    func=AF.Gelu)

        # ------- output projection (block pairs, N=512) -------
        with tc.tile_pool(name="yp", bufs=2) as yp, \
             tc.tile_pool(name="ap3", bufs=1, space="PSUM") as ap3:
            for p in range(NB // 2):
                y_ps = [ap3.tile([128, 512], f32, tag="y", bufs=4, name=f"y{cc}")
                        for cc in range(4)]
                for cc in range(4):
                    for ec in range(2):
                        nc.tensor.matmul(
                            out=y_ps[cc][:],
                            lhsT=wo_sb[:, ec, cc, :],
                            rhs=avn_sb[:, ec, p * 512:(p + 1) * 512],
                            start=(ec == 0), stop=(ec == 1))
                if has_mask:
                    mb_ps = ap3.tile([128, 512], f32, tag="mb", bufs=2, name="mb")
                    nc.tensor.matmul(
                        out=mb_ps[:], lhsT=ones_row[:],
                        rhs=mr_sb[:, p * 512:(p + 1) * 512],
                        start=True, stop=True)
                y_sb = yp.tile([128, 4, 512], f32, tag="ysb", name="y_sb")
                for cc in range(4):
                    if has_bias:
                        nc.scalar.activation(
                            out=y_sb[:, cc, :], in_=y_ps[cc][:],
                            func=AF.Identity, bias=bo_sb[:, cc:cc + 1])
                    else:
                        nc.vector.tensor_copy(y_sb[:, cc, :], y_ps[cc][:])
                    if has_mask:
                        nc.vector.tensor_mul(
                            y_sb[:, cc, :], y_sb[:, cc, :], mb_ps[:])
                    eng = nc.sync if cc < 2 else nc.gpsimd
                    eng.dma_start(
                        out=y_d[cc * 128:(cc + 1) * 128, p * 512:(p + 1) * 512],
                        in_=y_sb[:, cc, :])

    nc.compile()
    return nc


def get_program(has_bias: bool, has_mask: bool):
    key = (has_bias, has_mask)
    if key not in _PROG_CACHE:
        _PROG_CACHE[key] = _build_program(has_bias, has_mask)
    return _PROG_CACHE[key]


def _host_prep(x1, mask, Wq, bq, Wk, bk, Wv, bv, Wo, bo, has_bias, has_mask):
    """Build the per-core input maps (sharding + layout only)."""
    wq_t = np.ascontiguousarray(
        Wq.reshape(2, 128, 4, 128).transpose(3, 2, 0, 1))   # [p, cc, ec, f]
    wk_t = np.ascontiguousarray(
        Wk.reshape(2, 128, 4, 128).transpose(3, 2, 0, 1))
    wv_t = np.ascontiguousarray(
        Wv.reshape(256, 4, 128).transpose(2, 1, 0))          # [p, cc, e]
    wo_t = np.ascontiguousarray(
        Wo.reshape(4, 128, 2, 128).transpose(3, 2, 0, 1))    # [p, ec, cc, f]

    win = (np.arange(W) < W - 1).astype(np.float32)          # [512]
    in_maps = []
    for b in range(B):
        xp = np.pad(x1[b], ((0, 0), (HB, HB)))               # [C, S + 2HB]
        pmp = np.pad(mask[b, 0], (HB, HB))                   # [S + 2HB]
        for h in range(2):
            start = h * HALF
            x_halo = np.ascontiguousarray(xp[:, start:start + TH])
            # mask columns per local block
            lcol = np.empty((128, NB * 4), np.float32)
            fcol = np.empty((128, NB * 4), np.float32)
            for n in range(NB):
                gtok = start + n * D                         # padded-idx base
                pw = pmp[gtok:gtok + W].astype(np.float32)   # [512]
                f = (win * pw).astype(np.float32)
                lf = np.log(f + np.float32(1e-6)).astype(np.float32)
                fcol[:, n * 4:(n + 1) * 4] = f.reshape(4, 128).T
                lcol[:, n * 4:(n + 1) * 4] = lf.reshape(4, 128).T
            im = {
                "x_halo": x_halo, "wq_t": wq_t, "wk_t": wk_t,
                "wv_t": wv_t, "wo_t": wo_t, "lcol": lcol, "fcol": fcol,
                "onem": np.ones((128, 128), np.float32),
            }
            if has_bias or has_mask:
                im["oner"] = np.ones((1, 128), np.float32)
            if has_bias:
                im["bq2"] = np.ascontiguousarray(bq.reshape(2, 128).T)
                im["bk2"] = np.ascontiguousarray(bk.reshape(2, 128).T)
                im["bvr"] = np.ascontiguousarray(bv.reshape(1, 256))
                im["bo4"] = np.ascontiguousarray(bo.reshape(4, 128).T)
            if has_mask:
                im["mrow"] = np.ascontiguousarray(
                    mask[b, 0, start:start + HALF].reshape(1, HALF))
            in_maps.append(im)
    return in_maps


def kernel(x1, mask, Wq, bq, Wk, bk, Wv, bv, Wo, bo):
    global LAST_RESULT
    from concourse.bass_utils import run_bass_kernel_spmd

    x1 = np.asarray(x1, np.float32)
    mask = np.asarray(mask, np.float32)
    Wq, bq = np.asarray(Wq, np.float32), np.asarray(bq, np.float32)
    Wk, bk = np.asarray(Wk, np.float32), np.asarray(bk, np.float32)
    Wv, bv = np.asarray(Wv, np.float32), np.asarray(bv, np.float32)
    Wo, bo = np.asarray(Wo, np.float32), np.asarray(bo, np.float32)

    has_bias = bool(np.any(bq) or np.any(bk) or np.any(bv) or np.any(bo))
    has_mask = not bool(np.all(mask == 1.0))

    nc = get_program(has_bias, has_mask)
    in_maps = _host_prep(x1, mask, Wq, bq, Wk, bk, Wv, bv, Wo, bo,
                         has_bias, has_mask)
    res = run_bass_kernel_spmd(nc, in_maps, core_ids=list(range(NCORES)))
    LAST_RESULT = res

    y = np.empty((B, C, S), np.float32)
    for b in range(B):
        for h in range(2):
            y[b, :, h * HALF:(h + 1) * HALF] = res.results[b * 2 + h]["y"]
    return y


# revision 21
# speedup vs baseline: 1.4341x; 1.0018x over previous
"""Trainium2 Bass kernel for windowed (block-sparse) attention encoder.

Model (reference):
  q/k/v = 1x1 conv projections of x1 [B,C,S] with weights [E,C]
  queries split into nb = S/D blocks of D tokens; k/v use overlapping
  windows of width 2D (stride D, halo D/2 each side, zero-padded)
  attn = softmax(qk/sqrt(E) + log(fmask+1e-6)) * fmask
  y = Wo @ gelu(attn @ v) + bo, masked by the padding mask.

Sharding: 8 cores = batch (4) x sequence halves (2). Each core gets a
halo'd x slice [C, S/2 + D] so no cross-core communication is needed.

Device layout (per core):
  qw/kw: [E(part), token]   energyT[j,q] = kw^T qw  (j on partitions)
  vT:    [token(part), E]   av[e,q] = vT^T s2
  softmax runs along the partition (j) dim with no max subtraction
  (energies are O(+-10), exp is fp32-safe); the window/padding mask
  enters as a per-partition log-bias on the exp and a per-partition
  multiplier, so no partition-broadcasts are needed anywhere.
All matmul operands use float32r (fast fp32 path on the PE).
Projections and attention are interleaved per token tile so the PE
never drains between phases; gelu is fenced into its own region so the
ACT function-table set loads exactly once.
"""

import math
import os
from contextlib import ExitStack

import numpy as np

B, C, S = 4, 512, 8192
E, D = 256, 256
NCORES = 8
HALF = S // 2            # tokens per core
NB = HALF // D           # 16 blocks per core
HB = D // 2              # halo = 128
TH = HALF + 2 * HB       # halo'd token range = 4352
W = 2 * D                # window width 512

_PROG_CACHE = {}
LAST_RESULT = None


def _build_program(has_bias: bool, has_mask: bool):
    import concourse.tile as tile
    from concourse import bacc, mybir

    f32 = mybir.dt.float32
    fr = mybir.dt.float32r
    AF = mybir.ActivationFunctionType

    nc = bacc.Bacc("TRN2", target_bir_lowering=False, debug=False)

    x_d = nc.dram_tensor("x_halo", [C, TH], fr, kind="ExternalInput").ap()
    wq_d = nc.dram_tensor("wq_t", [128, 4, 2, 128], fr, kind="ExternalInput").ap()
    wk_d = nc.dram_tensor("wk_t", [128, 4, 2, 128], fr, kind="ExternalInput").ap()
    wv_d = nc.dram_tensor("wv_t", [128, 4, 256], fr, kind="ExternalInput").ap()
    wo_d = nc.dram_tensor("wo_t", [128, 2, 4, 128], fr, kind="ExternalInput").ap()
    lcol_d = nc.dram_tensor("lcol", [128, NB * 4], f32, kind="ExternalInput").ap()
    if has_mask:
        fcol_d = nc.dram_tensor("fcol", [128, NB * 4], f32,
                                kind="ExternalInput").ap()
    onem_d = nc.dram_tensor("onem", [128, 128], fr, kind="ExternalInput").ap()
    if has_bias or has_mask:
        oner_d = nc.dram_tensor("oner", [1, 128], fr, kind="ExternalInput").ap()
    if has_bias:
        bq_d = nc.dram_tensor("bq2", [128, 2], f32, kind="ExternalInput").ap()
        bk_d = nc.dram_tensor("bk2", [128, 2], f32, kind="ExternalInput").ap()
        bv_d = nc.dram_tensor("bvr", [1, 256], fr, kind="ExternalInput").ap()
        bo_d = nc.dram_tensor("bo4", [128, 4], f32, kind="ExternalInput").ap()
    if has_mask:
        mr_d = nc.dram_tensor("mrow", [1, HALF], fr, kind="ExternalInput").ap()
    y_d = nc.dram_tensor("y", [C, HALF], f32, kind="ExternalOutput").ap()

    with tile.TileContext(nc) as tc, ExitStack() as ctx:
        ctx.enter_context(nc.allow_low_precision(
            reason="float32r is 4-byte fp32-rounded; matmul accum stays fp32"))
        consts = ctx.enter_context(tc.tile_pool(name="consts", bufs=1))
        qkpool = ctx.enter_context(tc.tile_pool(name="qkpool", bufs=1))
        vtpool = ctx.enter_context(tc.tile_pool(name="vtpool", bufs=1))
        avpool = ctx.enter_context(tc.tile_pool(name="avpool", bufs=1))

        wq_sb = consts.tile([128, 4, 2, 128], fr)
        nc.scalar.dma_start(out=wq_sb[:], in_=wq_d[:])
        wk_sb = consts.tile([128, 4, 2, 128], fr)
        nc.scalar.dma_start(out=wk_sb[:], in_=wk_d[:])
        wv_sb = consts.tile([128, 4, 256], fr)
        nc.scalar.dma_start(out=wv_sb[:], in_=wv_d[:])
        lcol_sb = consts.tile([128, NB * 4], f32)
        nc.scalar.dma_start(out=lcol_sb[:], in_=lcol_d[:])
        if has_mask:
            fcol_sb = consts.tile([128, NB * 4], f32)
            nc.scalar.dma_start(out=fcol_sb[:], in_=fcol_d[:])
        ones_mat = consts.tile([128, 128], fr)
        nc.scalar.dma_start(out=ones_mat[:], in_=onem_d[:])
        if has_bias or has_mask:
            ones_row = consts.tile([1, 128], fr)
            nc.scalar.dma_start(out=ones_row[:], in_=oner_d[:])
        if has_bias:
            bq_sb = consts.tile([128, 2], f32)
            nc.scalar.dma_start(out=bq_sb[:], in_=bq_d[:])
            bk_sb = consts.tile([128, 2], f32)
            nc.scalar.dma_start(out=bk_sb[:], in_=bk_d[:])
            bv_sb = consts.tile([1, 256], fr)
            nc.scalar.dma_start(out=bv_sb[:], in_=bv_d[:])
            bo_sb = consts.tile([128, 4], f32)
            nc.scalar.dma_start(out=bo_sb[:], in_=bo_d[:])
        if has_mask:
            mr_sb = consts.tile([1, HALF], fr)
            nc.scalar.dma_start(out=mr_sb[:], in_=mr_d[:])

        # persistent projections (cover the full halo'd range)
        qw_sb = [qkpool.tile([128, TH], fr, name=f"qw{ec}") for ec in range(2)]
        kw_sb = [qkpool.tile([128, TH], fr, name=f"kw{ec}") for ec in range(2)]
        vt_sb = vtpool.tile([128, TH // 128, 256], fr)  # [tok%128, tokchunk, e]
        avn_sb = avpool.tile([128, 2, NB * 256], fr)    # pre-gelu normalized av

        def emit_attention(n, sp, ps):
            base = n * 256
            e_ps = [ps.tile([128, 256], f32, tag="e", bufs=2, name=f"e{jc}")
                    for jc in range(4)]
            for jc in range(4):
                for ec in range(2):
                    nc.tensor.matmul(
                        out=e_ps[jc][:],
                        lhsT=kw_sb[ec][:, base + jc * 128:base + (jc + 1) * 128],
                        rhs=qw_sb[ec][:, HB + base:HB + base + 256],
                        start=(ec == 0), stop=(ec == 1))
            s_t = sp.tile([128, 4, 256], fr, tag="s", name="s_t")
            for jc in range(4):
                nc.scalar.activation(
                    out=s_t[:, jc, :], in_=e_ps[jc][:], func=AF.Exp,
                    bias=lcol_sb[:, n * 4 + jc:n * 4 + jc + 1],
                    scale=1.0 / math.sqrt(E))
            # zb[p, q] = sum_j s~[j, q]: pairwise DVE partial sums, then
            # one ones^T matmul to reduce across the remaining partition dim
            sp2 = sp.tile([128, 2, 256], fr, tag="sp2", bufs=1, name="sp2")
            nc.vector.tensor_add(sp2[:], s_t[:, 0:2, :], s_t[:, 2:4, :])
            ssum = sp.tile([128, 256], fr, tag="ssum", bufs=1, name="ssum")
            nc.vector.tensor_add(ssum[:], sp2[:, 0, :], sp2[:, 1, :])
            zb_ps = ps.tile([128, 256], f32, tag="zz", bufs=1, name="zb_ps")
            nc.tensor.matmul(out=zb_ps[:], lhsT=ones_mat[:], rhs=ssum[:],
                             start=True, stop=True)
            zscr = sp.tile([128, 256], f32, tag="zscr", bufs=1, name="zscr")
            zrec = sp.tile([128, 256], f32, tag="zrec", bufs=1, name="zrec")
            nc.vector.reciprocal_approx_accurate(
                out=zrec[:], in_=zb_ps[:], scratch=zscr[:])
            if has_mask:
                # general float mask: apply the post-softmax fmask factor
                s2_t = sp.tile([128, 4, 256], fr, tag="s2", bufs=1, name="s2_t")
                for jc in range(4):
                    nc.vector.tensor_scalar_mul(
                        s2_t[:, jc, :], s_t[:, jc, :],
                        fcol_sb[:, n * 4 + jc:n * 4 + jc + 1])
                av_rhs = s2_t
            else:
                # all-ones mask: fmask only zeroes columns whose exp is
                # already scaled by 1e-6 via the log bias (<=3e-6 relative
                # contribution) -- skip the multiply, use s~ directly
                av_rhs = s_t
            av_ps = [ps.tile([128, 256], f32, tag="av", bufs=2, name=f"av{ec}")
                     for ec in range(2)]
            for ec in range(2):
                for jc in range(4):
                    nc.tensor.matmul(
                        out=av_ps[ec][:],
                        lhsT=vt_sb[:, 2 * n + jc, ec * 128:(ec + 1) * 128],
                        rhs=av_rhs[:, jc, :],
                        start=(jc == 0), stop=(jc == 3))
            for ec in range(2):
                nc.vector.tensor_mul(
                    avn_sb[:, ec, n * 256:(n + 1) * 256],
                    av_ps[ec][:], zrec[:])

        # ---- interleaved projections + attention ----
        tts = [(i * 512, 512) for i in range(8)] + [(4096, 256)]
        next_blk = 0
        with tc.tile_pool(name="xp", bufs=2) as xp, \
             tc.tile_pool(name="sp", bufs=2) as sp, \
             tc.tile_pool(name="ps", bufs=1, space="PSUM") as ps:
            for (t0, tw) in tts:
                x_t = [xp.tile([128, 512], fr, tag=f"x{cc}", name=f"x{cc}")
                       for cc in range(4)]
                for cc in range(4):
                    eng = nc.sync if cc < 2 else nc.gpsimd
                    eng.dma_start(
                        out=x_t[cc][:, :tw],
                        in_=x_d[cc * 128:(cc + 1) * 128, t0:t0 + tw])
                # q/k projections: [E, token] layout
                for (w_sb, b_sb, out_sb) in (
                    (wq_sb, "bq", qw_sb), (wk_sb, "bk", kw_sb)):
                    for ec in range(2):
                        qk_ps = ps.tile([128, 512], f32, tag="qk", bufs=2,
                                        name="qk_ps")
                        for cc in range(4):
                            nc.tensor.matmul(
                                out=qk_ps[:, :tw],
                                lhsT=w_sb[:, cc, ec, :],
                                rhs=x_t[cc][:, :tw],
                                start=(cc == 0), stop=(cc == 3))
                        if has_bias:
                            bias = (bq_sb if b_sb == "bq" else bk_sb)[:, ec:ec + 1]
                            nc.scalar.activation(
                                out=out_sb[ec][:, t0:t0 + tw], in_=qk_ps[:, :tw],
                                func=AF.Identity, bias=bias)
                        else:
                            nc.scalar.copy(
                                out=out_sb[ec][:, t0:t0 + tw], in_=qk_ps[:, :tw])
                # vT projection: [token, E] layout
                for tci in range(tw // 128):
                    g = t0 // 128 + tci
                    vp = ps.tile([128, 256], f32, tag="v", bufs=1, name="vps")
                    for cc in range(4):
                        nc.tensor.matmul(
                            out=vp[:],
                            lhsT=x_t[cc][:, tci * 128:(tci + 1) * 128],
                            rhs=wv_sb[:, cc, :],
                            start=(cc == 0), stop=(cc == 3 and not has_bias))
                    if has_bias:
                        nc.tensor.matmul(
                            out=vp[:], lhsT=ones_row[:], rhs=bv_sb[:],
                            start=False, stop=True)
                    nc.vector.tensor_copy(vt_sb[:, g, :], vp[:])
                # attention for every block whose window is now projected
                while next_blk < NB and next_blk * 256 + 512 <= t0 + tw:
                    emit_attention(next_blk, sp, ps)
                    next_blk += 1
                if t0 == 0:
                    # Wo is first needed in the output projection; keep it
                    # off the head of the weight-DMA queue
                    wo_sb = consts.tile([128, 2, 4, 128], fr)
                    nc.scalar.dma_start(out=wo_sb[:], in_=wo_d[:])

        # ---- gelu, fenced off so the ACT table set loads once ----
        tc.no_sync_barrier()
        for p in range(NB // 2):
            for ec in range(2):
                nc.scalar.activation(
                    out=avn_sb[:, ec, p * 512:(p + 1) * 512],
                    in_=avn_sb[:, ec, p * 512:(p + 1) * 512],
                    func=AF.Gelu)

        # ------- output projection (block pairs, N=512) -------
        with tc.tile_pool(name="yp", bufs=2) as yp, \
             tc.tile_pool(name="ap3", bufs=1, space="PSUM") as ap3:
            for p in range(NB // 2):
                y_ps = [ap3.tile([128, 512], f32, tag="y", bufs=4, name=f"y{cc}")
                        for cc in range(4)]
                for cc in range(4):
                    for ec in range(2):
                        nc.tensor.matmul(
                            out=y_ps[cc][:],
                            lhsT=wo_sb[:, ec, cc, :],
                            rhs=avn_sb[:, ec, p * 512:(p + 1) * 512],
                            start=(ec == 0), stop=(ec == 1))
                if has_mask:
                    mb_ps = ap3.tile([128, 512], f32, tag="mb", bufs=2, name="mb")
                    nc.tensor.matmul(
                        out=mb_ps[:], lhsT=ones_row[:],
                        rhs=mr_sb[:, p * 512:(p + 1) * 512],
                        start=True, stop=True)
                y_sb = yp.tile([128, 4, 512], f32, tag="ysb", name="y_sb")
                for cc in range(4):
                    if has_bias:
                        nc.scalar.activation(
                            out=y_sb[:, cc, :], in_=y_ps[cc][:],
                            func=AF.Identity, bias=bo_sb[:, cc:cc + 1])
                    else:
                        nc.vector.tensor_copy(y_sb[:, cc, :], y_ps[cc][:])
                    if has_mask:
                        nc.vector.tensor_mul(
                            y_sb[:, cc, :], y_sb[:, cc, :], mb_ps[:])
                    eng = nc.sync if cc < 2 else nc.gpsimd
                    eng.dma_start(
                        out=y_d[cc * 128:(cc + 1) * 128, p * 512:(p + 1) * 512],
                        in_=y_sb[:, cc, :])

    nc.compile()
    return nc


def get_program(has_bias: bool, has_mask: bool):
    key = (has_bias, has_mask)
    if key not in _PROG_CACHE:
        _PROG_CACHE[key] = _build_program(has_bias, has_mask)
    return _PROG_CACHE[key]


def _host_prep(x1, mask, Wq, bq, Wk, bk, Wv, bv, Wo, bo, has_bias, has_mask):
    """Build the per-core input maps (sharding + layout only)."""
    wq_t = np.ascontiguousarray(
        Wq.reshape(2, 128, 4, 128).transpose(3, 2, 0, 1))   # [p, cc, ec, f]
    wk_t = np.ascontiguousarray(
        Wk.reshape(2, 128, 4, 128).transpose(3, 2, 0, 1))
    wv_t = np.ascontiguousarray(
        Wv.reshape(256, 4, 128).transpose(2, 1, 0))          # [p, cc, e]
    wo_t = np.ascontiguousarray(
        Wo.reshape(4, 128, 2, 128).transpose(3, 2, 0, 1))    # [p, ec, cc, f]

    win = (np.arange(W) < W - 1).astype(np.float32)          # [512]
    in_maps = []
    for b in range(B):
        xp = np.pad(x1[b], ((0, 0), (HB, HB)))               # [C, S + 2HB]
        pmp = np.pad(mask[b, 0], (HB, HB))                   # [S + 2HB]
        for h in range(2):
            start = h * HALF
            x_halo = np.ascontiguousarray(xp[:, start:start + TH])
            # mask columns per local block
            lcol = np.empty((128, NB * 4), np.float32)
            fcol = np.empty((128, NB * 4), np.float32)
            for n in range(NB):
                gtok = start + n * D                         # padded-idx base
                pw = pmp[gtok:gtok + W].astype(np.float32)   # [512]
                f = (win * pw).astype(np.float32)
                lf = np.log(f + np.float32(1e-6)).astype(np.float32)
                fcol[:, n * 4:(n + 1) * 4] = f.reshape(4, 128).T
                lcol[:, n * 4:(n + 1) * 4] = lf.reshape(4, 128).T
            im = {
                "x_halo": x_halo, "wq_t": wq_t, "wk_t": wk_t,
                "wv_t": wv_t, "wo_t": wo_t, "lcol": lcol,
                "onem": np.ones((128, 128), np.float32),
            }
            if has_mask:
                im["fcol"] = fcol
            if has_bias or has_mask:
                im["oner"] = np.ones((1, 128), np.float32)
            if has_bias:
                im["bq2"] = np.ascontiguousarray(bq.reshape(2, 128).T)
                im["bk2"] = np.ascontiguousarray(bk.reshape(2, 128).T)
                im["bvr"] = np.ascontiguousarray(bv.reshape(1, 256))
                im["bo4"] = np.ascontiguousarray(bo.reshape(4, 128).T)
            if has_mask:
                im["mrow"] = np.ascontiguousarray(
                    mask[b, 0, start:start + HALF].reshape(1, HALF))
            in_maps.append(im)
    return in_maps


def kernel(x1, mask, Wq, bq, Wk, bk, Wv, bv, Wo, bo):
    global LAST_RESULT
    from concourse.bass_utils import run_bass_kernel_spmd

    x1 = np.asarray(x1, np.float32)
    mask = np.asarray(mask, np.float32)
    Wq, bq = np.asarray(Wq, np.float32), np.asarray(bq, np.float32)
    Wk, bk = np.asarray(Wk, np.float32), np.asarray(bk, np.float32)
    Wv, bv = np.asarray(Wv, np.float32), np.asarray(bv, np.float32)
    Wo, bo = np.asarray(Wo, np.float32), np.asarray(bo, np.float32)

    has_bias = bool(np.any(bq) or np.any(bk) or np.any(bv) or np.any(bo))
    has_mask = not bool(np.all(mask == 1.0))

    nc = get_program(has_bias, has_mask)
    in_maps = _host_prep(x1, mask, Wq, bq, Wk, bk, Wv, bv, Wo, bo,
                         has_bias, has_mask)
    res = run_bass_kernel_spmd(nc, in_maps, core_ids=list(range(NCORES)))
    LAST_RESULT = res

    y = np.empty((B, C, S), np.float32)
    for b in range(B):
        for h in range(2):
            y[b, :, h * HALF:(h + 1) * HALF] = res.results[b * 2 + h]["y"]
    return y
